# revision 24
# baseline (speedup 1.0000x reference)
"""Trainium2 Bass kernel for one pre-LN transformer block (B=8, T=1024, C=256,
H=16 heads of size 16, FFN 256->1024->256), data-parallel over batch across 8
NeuronCores (one batch element per core).

v2 changes vs baseline (175-205us):
  - causal mask applied as a bitwise-AND on the bf16 *bits* of the exp'd
    scores (i32-packed keep-mask), one DVE op per diag tile covering all 4
    heads -- replaces the per-head strided multiply pass
  - exp evacuation split between ScalarE (true Exp) and VectorE (Schraudolph
    bits) per head-pair via a greedy runtime-cost balancer; all other PSUM
    evacuations (QKV copies, FFN1 relu, transpose copies) also greedy-balanced
    across the two PSUM-capable engines
  - PSUM tags: sps(4 banks) / pv(2) / mm(2, everything else) so attention and
    FFN never contend for banks
  - x DMA'd per-tile so LN1 starts after the first 128 rows land; weight DMAs
    moved to the gpsimd queue in first-use order
  - attention chunk-1 units interleaved between FFN chunk-0 stages to keep the
    PE array busy (HAM clock gate) through the whole span
"""

import os
import sys

for _p in ("/opt/trn_rl_repo", "/root/.axon_site/_ro/trn_rl_repo"):
    if os.path.isdir(_p) and _p not in sys.path:
        sys.path.append(_p)

import numpy as np
import ml_dtypes

# problem shapes (hardcoded per contest rules)
B, T, C, H, D, F = 8, 1024, 256, 16, 16, 1024
P = 128          # partitions
NT = T // P      # 8 T-tiles
HP = 32          # padded per-head stride (Q/K/V/out layouts)
CP = H * HP      # 512 padded channel dim
NPACK = 4        # head packs (4 heads per 128-partition tile)
NKC = C // P     # 2 k-tiles over C
EPS = 1e-5
SCALE = D ** -0.5
MAGIC = 0x5F3759DF
# Schraudolph-style exp to bf16 bits: bf16_bits(exp(SCALE*s)) ~= EXP_A*s + EXP_B
EXP_A = (2 ** 7) * SCALE * 1.4426950408889634
EXP_B = 2 ** 7 * 127 - 5.6

_BF16 = ml_dtypes.bfloat16

_cache = {}


def _build_program(qk_bias=False, bp_zero=False, b2_zero=False):
    import concourse.bass as bass
    import concourse.bacc as bacc
    import concourse.tile as tile
    import concourse.mybir as mybir

    dt = mybir.dt
    f32, bf16, i32, i16 = dt.float32, dt.bfloat16, dt.int32, dt.int16
    AF = mybir.ActivationFunctionType
    ALU = mybir.AluOpType

    nc = bacc.Bacc("TRN2", target_bir_lowering=False, debug=False)

    # ---- DRAM I/O ----
    x_d = nc.dram_tensor("x", [T, C], f32, kind="ExternalInput")
    wq_d = nc.dram_tensor("wq", [C, CP], bf16, kind="ExternalInput")
    wk_d = nc.dram_tensor("wk", [C, CP], bf16, kind="ExternalInput")
    wv_d = nc.dram_tensor("wv", [C, CP], bf16, kind="ExternalInput")
    wp_d = nc.dram_tensor("wp", [CP, C], bf16, kind="ExternalInput")
    w1_d = nc.dram_tensor("w1", [C, F], bf16, kind="ExternalInput")
    w2_d = nc.dram_tensor("w2", [F, C], bf16, kind="ExternalInput")
    bq_d = nc.dram_tensor("bq", [CP], f32, kind="ExternalInput")
    bk_d = nc.dram_tensor("bk", [CP], f32, kind="ExternalInput")
    bp_d = nc.dram_tensor("bprow", [C], f32, kind="ExternalInput")
    b1_d = nc.dram_tensor("b1p", [F], f32, kind="ExternalInput")
    b2_d = nc.dram_tensor("b2row", [C], f32, kind="ExternalInput")
    out_d = nc.dram_tensor("out", [T, C], f32, kind="ExternalOutput")

    ident_np = np.eye(P, dtype=_BF16)
    # causal keep-mask for a diagonal S^T tile, packed as i32 over bf16 pairs:
    # partition = tk local, i32 word w covers tq columns 2w (low16) / 2w+1
    # (high16); keep (all-ones halfword) iff tq >= tk.
    keep = (np.arange(P)[None, :] >= np.arange(P)[:, None])  # [tk, tq]
    lo = keep[:, 0::2].astype(np.uint32) * 0x0000FFFF
    hi = keep[:, 1::2].astype(np.uint32) * 0xFFFF0000
    triw_np = (lo | hi).astype(np.uint32).view(np.int32)  # [128, 64]
    triw4_np = np.tile(triw_np[:, None, :], (1, NPACK, 1))  # [128, 4, 64]
    trib4_np = np.tile(
        keep.astype(_BF16)[:, None, :], (1, NPACK, 1))  # [128, 4, 128] bf16
    ident_d = nc.inline_tensor(ident_np, name="ident")
    triw4_d = nc.inline_tensor(triw4_np, name="triw4")
    trib4_d = nc.inline_tensor(trib4_np, name="trib4")

    # greedy engine-load balancer for PSUM-evacuation ops
    eng_t = {"sc": 0.0, "dv": 0.0}

    def pick_engine(fd):
        cs = 0.84 * fd + 300.0
        cd = 1.05 * fd + 130.0
        if eng_t["sc"] + cs <= eng_t["dv"] + cd:
            eng_t["sc"] += cs
            return "sc"
        eng_t["dv"] += cd
        return "dv"

    def dv_fixed(fd, ovh=130.0):
        eng_t["dv"] += 1.05 * fd + ovh

    with tile.TileContext(nc) as tc:
        consts = tc.alloc_tile_pool(name="consts", bufs=1)
        data = tc.alloc_tile_pool(name="data", bufs=1)
        attn = tc.alloc_tile_pool(name="attn", bufs=1)
        work = tc.alloc_tile_pool(name="work", bufs=4)
        psum = tc.alloc_tile_pool(name="psum", bufs=1, space="PSUM")

        # ---- persistent SBUF tensors ----
        ident_s = consts.tile([P, P], bf16)
        triw4_s = consts.tile([P, NPACK, 64], i32)
        trib4_s = consts.tile([P, NPACK, P], bf16)
        wq_s = consts.tile([P, NKC, CP], bf16)
        wk_s = consts.tile([P, NKC, CP], bf16)
        wv_s = consts.tile([P, NKC, CP], bf16)
        wp_s = consts.tile([P, NPACK, C], bf16)
        w1_s = consts.tile([P, NKC, F], bf16)
        w2_s = consts.tile([P, NT, C], bf16)
        bq_s = consts.tile([P, NPACK], f32)
        bk_s = consts.tile([P, NPACK], f32)
        b1_s = consts.tile([P, NT], f32)

        xs = data.tile([P, NT, C], f32)
        xbp = xs if bp_zero else data.tile([P, NT, C], f32)
        h1T = data.tile([P, NKC, T], bf16)
        QT = data.tile([P, NPACK, T], bf16)
        KT = data.tile([P, NPACK, T], bf16)
        Vv = data.tile([P, NT, CP], bf16)
        OUTT = data.tile([P, NPACK, T], bf16)
        x1 = data.tile([P, NT, C], f32)
        x1b = x1 if b2_zero else data.tile([P, NT, C], f32)
        h2T = data.tile([P, NKC, T], bf16)
        HT = data.tile([P, NT, F], bf16)

        # ---- input DMAs: x tiles first, round-robined over the three DMA
        # queues so the first LN tiles land asap; weights follow in use order
        x_r = x_d[:, :].rearrange("(j p) c -> p j c", p=P)
        xq = [nc.sync, nc.scalar, nc.gpsimd]
        for j in range(NT):
            xq[j % 3].dma_start(out=xs[:, j], in_=x_r[:, j])
        nc.sync.dma_start(out=ident_s, in_=ident_d[:, :])
        nc.scalar.dma_start(out=triw4_s, in_=triw4_d[:, :, :])
        nc.scalar.dma_start(out=trib4_s, in_=trib4_d[:, :, :])
        nc.gpsimd.dma_start(out=wq_s, in_=wq_d[:, :].rearrange("(k p) c -> p k c", p=P))
        nc.gpsimd.dma_start(out=wk_s, in_=wk_d[:, :].rearrange("(k p) c -> p k c", p=P))
        nc.gpsimd.dma_start(out=wv_s, in_=wv_d[:, :].rearrange("(k p) c -> p k c", p=P))
        nc.sync.dma_start(out=wp_s, in_=wp_d[:, :].rearrange("(k p) c -> p k c", p=P))
        nc.sync.dma_start(out=w1_s, in_=w1_d[:, :].rearrange("(k p) c -> p k c", p=P))
        nc.sync.dma_start(out=w2_s, in_=w2_d[:, :].rearrange("(k p) c -> p k c", p=P))
        if qk_bias:
            nc.sync.dma_start(out=bq_s, in_=bq_d[:].rearrange("(m p) -> p m", p=P))
            nc.sync.dma_start(out=bk_s, in_=bk_d[:].rearrange("(m p) -> p m", p=P))
        nc.sync.dma_start(out=b1_s, in_=b1_d[:].rearrange("(m p) -> p m", p=P))
        if not bp_zero:
            bp_b = bass.AP(tensor=bp_d, offset=0, ap=[[0, P], [1, C]])
            bpt = consts.tile([P, C], f32)
            nc.sync.dma_start(out=bpt, in_=bp_b)
            for j in range(NT):
                nc.sync.dma_start(out=xbp[:, j], in_=x_r[:, j])
                nc.vector.tensor_add(out=xbp[:, j], in0=xbp[:, j], in1=bpt)
        if not b2_zero:
            b2t = consts.tile([P, C], f32)
            b2_b = bass.AP(tensor=b2_d, offset=0, ap=[[0, P], [1, C]])
            nc.sync.dma_start(out=b2t, in_=b2_b)

        def evac_copy(dst, src, fd):
            """PSUM->SBUF copy on whichever engine is less loaded."""
            if pick_engine(fd) == "sc":
                nc.scalar.copy(dst, src)
            else:
                nc.vector.tensor_copy(dst, src)

        def ln_phase(src, dst_hT, tag, tiles, dma_tp=False):
            """LayerNorm the given tiles of src [128, 8, 256] f32 and write
            the transposed bf16 result into dst_hT [128, 2, 1024].
            dma_tp: transpose via the DMA xbar (no PE / PSUM / evac) --
            preferred mid-kernel when the PE array is busy."""
            nj = len(tiles)
            mvall = work.tile([P, nj, 2], f32, tag="mvall", name=f"mv_{tag}")
            for jx, j in enumerate(tiles):
                stats = work.tile([P, 6], f32, tag="stats")
                nc.vector.bn_stats(out=stats, in_=src[:, j])
                nc.vector.bn_aggr(out=mvall[:, jx], in_=stats)
            dv_fixed(nj * 256 + nj * 8, nj * 300.0)
            # rstd for all tiles: Quake rsqrt + 2 Newton steps (pure DVE)
            vpe = work.tile([P, nj], f32, tag="vpe", name=f"vpe_{tag}")
            nc.vector.tensor_scalar_add(out=vpe, in0=mvall[:, :, 1], scalar1=EPS)
            sh = work.tile([P, nj], i32, tag="rsq_sh")
            nc.vector.tensor_scalar(
                out=sh, in0=vpe.bitcast(i32), scalar1=1, scalar2=None,
                op0=ALU.logical_shift_right,
            )
            y0 = work.tile([P, nj], i32, tag="rsq_y0")
            nc.vector.tensor_scalar(
                out=y0, in0=sh, scalar1=-1, scalar2=MAGIC,
                op0=ALU.mult, op1=ALU.add,
            )
            y = y0.bitcast(f32)
            rsq = work.tile([P, nj], f32, tag="rsq", name=f"rsq_{tag}")
            tmp = work.tile([P, nj], f32, tag="rsq_tmp")
            for it in range(2):
                nc.vector.tensor_tensor(out=tmp, in0=y, in1=y, op=ALU.mult)
                nc.vector.tensor_tensor(out=tmp, in0=tmp, in1=vpe, op=ALU.mult)
                nc.vector.tensor_scalar(
                    out=tmp, in0=tmp, scalar1=-0.5, scalar2=1.5,
                    op0=ALU.mult, op1=ALU.add,
                )
                nc.vector.tensor_tensor(out=rsq, in0=tmp, in1=y, op=ALU.mult)
                y = rsq
            dv_fixed(nj * 9, 9 * 130.0)
            for jx, j in enumerate(tiles):
                hs = work.tile([P, C], bf16, tag="hstraight")
                nc.vector.tensor_scalar(
                    out=hs, in0=src[:, j],
                    scalar1=mvall[:, jx, 0:1], scalar2=rsq[:, jx : jx + 1],
                    op0=ALU.subtract, op1=ALU.mult,
                )
                dv_fixed(128)
                if dma_tp:
                    nc.sync.dma_start(
                        out=dst_hT[:, :, j * P : (j + 1) * P], in_=hs,
                        transpose=True,
                    )
                else:
                    tp = psum.tile([P, 2, P], bf16, tag="big", bufs=3)
                    nc.tensor.transpose(tp[:, 0], hs[:, 0:P], ident_s)
                    nc.tensor.transpose(tp[:, 1], hs[:, P : 2 * P], ident_s)
                    evac_copy(dst_hT[:, :, j * P : (j + 1) * P], tp, 256)

        # ---- Q^T / K^T (padded layout, bias folded in evac) ----
        def qk_chunk(c):
            for (name, w_s, b_s, dstT) in (("q", wq_s, bq_s, QT), ("k", wk_s, bk_s, KT)):
                for m in range(NPACK):
                    ps = psum.tile([P, 512], f32, tag="big", bufs=3)
                    for k in range(NKC):
                        nc.tensor.matmul(
                            ps,
                            lhsT=w_s[:, k, m * P : (m + 1) * P],
                            rhs=h1T[:, k, c * 512 : (c + 1) * 512],
                            start=(k == 0), stop=(k == NKC - 1),
                        )
                    dst = dstT[:, m, c * 512 : (c + 1) * 512]
                    if qk_bias:
                        nc.vector.tensor_scalar_add(
                            out=dst, in0=ps, scalar1=b_s[:, m : m + 1])
                        dv_fixed(512)
                    else:
                        evac_copy(dst, ps, 512)

        # ---- V (straight, padded 32-wide blocks; col 16 of each = ones) ----
        def v_tiles(js):
            for j in js:
                ps = psum.tile([P, 512], f32, tag="big", bufs=3)
                for k in range(NKC):
                    nc.tensor.matmul(
                        ps,
                        lhsT=h1T[:, k, j * P : (j + 1) * P],
                        rhs=wv_s[:, k, :],
                        start=(k == 0), stop=(k == NKC - 1),
                    )
                evac_copy(Vv[:, j, :], ps, 512)
            ones_cols = Vv.rearrange("p j (h e) -> p j h e", e=HP)[
                :, js[0] : js[-1] + 1, :, 16:17]
            nc.gpsimd.memset(ones_cols, 1.0)

        # ---- attention: unit = (tq-chunk, pack) ----
        pend = []

        def tick():
            if pend:
                pend.pop(0)()

        def attn_unit(p, cj):
            nt_c = 4 * cj + 4
            expc = attn.tile([P, NPACK, nt_c, 512], bf16, tag=f"expc{cj}",
                             bufs=2, name=f"expc{p}_{cj}")
            tiles = list(range(0, min(NT, 4 * cj + 4)))
            # S^T as 32x32 subarray tiles; 2 heads share one 2-bank psum tile;
            # exp evac engine chosen per head-pair by the load balancer
            def exp_evac(dst, src, fd, eng):
                if eng == "sc":
                    nc.scalar.activation(
                        out=dst, in_=src, func=AF.Exp, scale=SCALE)
                else:
                    nc.vector.tensor_scalar(
                        out=dst.bitcast(i16), in0=src,
                        scalar1=EXP_A, scalar2=EXP_B,
                        op0=ALU.mult, op1=ALU.add,
                    )

            state = {"pv": None}
            for i in tiles:
                off = max(0, P * i - 512 * cj)  # valid start within chunk
                n = 512 - off
                # head pairs: greedy engine pick with a small penalty for
                # putting both pairs of one i-tile on the same engine (keeps
                # the two exps concurrent without forcing a 50/50 load split)
                prev_eng = None
                for q in range(2):
                    sp = psum.tile([P, 2, 512], f32, tag="big", bufs=3,
                                   name=f"sp{p}_{cj}_{i}_{q}")
                    for e in range(2):
                        hh = 2 * q + e
                        nc.tensor.matmul(
                            sp[:, e, 0:n],
                            lhsT=KT[HP * hh : HP * (hh + 1), p,
                                    i * P : (i + 1) * P],
                            rhs=QT[HP * hh : HP * (hh + 1), p,
                                   512 * cj + off : 512 * cj + off + n],
                            start=True, stop=True,
                            tile_position=(HP * hh, 0),
                        )
                    if prev_eng is not None:
                        eng_t[prev_eng] += 500.0
                        eng = pick_engine(2 * n)
                        eng_t[prev_eng] -= 500.0
                    else:
                        eng = pick_engine(2 * n)
                    prev_eng = eng
                    exp_evac(expc[:, 2 * q : 2 * q + 2, i, off : off + n],
                             sp[:, :, 0:n], 2 * n, eng)
                # causal mask on the diagonal tile: 0/1 bf16 multiply on the
                # otherwise-idle GpSimd -- PV now runs a full unit later, so
                # the longer GpSimd latency is completely hidden
                if 4 * cj <= i < 4 * cj + 4:
                    od = P * i - 512 * cj
                    eb = expc[:, :, i, od : od + P]
                    nc.gpsimd.tensor_tensor(
                        out=eb, in0=eb, in1=trib4_s, op=ALU.mult)
                # let one deferred PV step of the previous unit run between
                # this unit's S-tiles to keep the PE array streaming
                if pend:
                    pend.pop(0)()

            # deferred PV + normalize: emitted as thunks interleaved into the
            # NEXT unit's S-phase (keeps PE busy, hides mask latency)
            last = max(tiles)

            def pv_step(i):
                off = max(0, P * i - 512 * cj)
                n = 512 - off
                if state["pv"] is None:
                    state["pv"] = psum.tile([P, 512], f32, tag="pv", bufs=2,
                                            name=f"pv{p}_{cj}")
                for hh in range(NPACK):
                    h = 4 * p + hh
                    nc.tensor.matmul(
                        state["pv"][HP * hh : HP * (hh + 1), off : off + n],
                        lhsT=Vv[:, i, HP * h : HP * (h + 1)],
                        rhs=expc[:, hh, i, off : off + n],
                        start=(i == 0), stop=(i == last),
                        tile_position=(0, HP * hh),
                        skip_group_check=True,
                    )

            def norm_step():
                pv = state["pv"]
                zbc = work.tile([P, 512], f32, tag="zbc", bufs=2)
                rz = work.tile([P, 512], f32, tag="rz", bufs=2)
                nc.vector.stream_shuffle(zbc, pv, mask=[16] * 32)
                nc.vector.reciprocal_approx_fast(out=rz, in_=zbc)
                nc.vector.tensor_tensor(
                    out=OUTT[:, p, 512 * cj : 512 * (cj + 1)], in0=pv,
                    in1=rz, op=ALU.mult,
                )
                dv_fixed(3 * 512, 3 * 200.0)

            return [lambda i=i: pv_step(i) for i in tiles] + [norm_step]

        def proj_tile(j):
            ps = psum.tile([P, C], f32, tag="big", bufs=3)
            for k in range(NPACK):
                nc.tensor.matmul(
                    ps,
                    lhsT=OUTT[:, k, j * P : (j + 1) * P],
                    rhs=wp_s[:, k, :],
                    start=(k == 0), stop=(k == NPACK - 1),
                )
            nc.vector.tensor_add(out=x1[:, j], in0=ps, in1=xbp[:, j])
            dv_fixed(256)
            if not b2_zero:
                nc.vector.tensor_add(out=x1b[:, j], in0=x1[:, j], in1=b2t)
                dv_fixed(256)

        def ffn1_tiles(c, fs):
            for f in fs:
                tick()
                ps = psum.tile([P, 512], f32, tag="big", bufs=3)
                for k in range(NKC):
                    nc.tensor.matmul(
                        ps,
                        lhsT=w1_s[:, k, f * P : (f + 1) * P],
                        rhs=h2T[:, k, c * 512 : (c + 1) * 512],
                        start=(k == 0), stop=(k == NKC - 1),
                    )
                dst = HT[:, f, c * 512 : (c + 1) * 512]
                if pick_engine(512) == "sc":
                    nc.scalar.activation(
                        out=dst, in_=ps, func=AF.Relu, bias=b1_s[:, f : f + 1])
                else:
                    nc.vector.tensor_scalar(
                        out=dst, in0=ps, scalar1=b1_s[:, f : f + 1], scalar2=0.0,
                        op0=ALU.add, op1=ALU.max,
                    )

        def ffn2_tile(j):
            ps = psum.tile([P, C], f32, tag="big", bufs=3)
            for f in range(NT):
                nc.tensor.matmul(
                    ps,
                    lhsT=HT[:, f, j * P : (j + 1) * P],
                    rhs=w2_s[:, f, :],
                    start=(f == 0), stop=(f == NT - 1),
                )
            outs = work.tile([P, C], f32, tag="outs", bufs=2)
            nc.vector.tensor_add(out=outs, in0=ps, in1=x1b[:, j])
            dv_fixed(256)
            nc.sync.dma_start(
                out=out_d[:, :].rearrange("(t p) c -> p t c", p=P)[:, j], in_=outs
            )

        # ---- schedule: attention units start as soon as their inputs
        # exist and stay interleaved with LN/QKV/FFN so the exp engines are
        # never starved and the PE always has ready work ----
        ln_phase(xs, h1T, "ln1a0", [0, 1])
        ln_phase(xs, h1T, "ln1a1", [2, 3])
        qk_chunk(0)
        v_tiles([0, 1, 2, 3])
        ln_phase(xs, h1T, "ln1b0", [4, 5])
        ln_phase(xs, h1T, "ln1b1", [6, 7])
        def run_unit(p, cj):
            new_thunks = attn_unit(p, cj)
            pend.extend(new_thunks)

        run_unit(0, 0)
        qk_chunk(1)
        run_unit(1, 0)
        v_tiles([4, 5, 6, 7])
        run_unit(2, 0)
        run_unit(3, 0)
        run_unit(0, 1)
        for j in range(4):
            proj_tile(j)
            tick()
        run_unit(1, 1)
        ln_phase(x1, h2T, "ln2a", [0, 1, 2, 3], dma_tp=True)
        run_unit(2, 1)
        ffn1_tiles(0, [0, 1, 2, 3])
        run_unit(3, 1)
        ffn1_tiles(0, [4, 5, 6, 7])
        while pend:
            tick()
        for j in range(4, 8):
            proj_tile(j)
        for j in range(4):
            ffn2_tile(j)
        ln_phase(x1, h2T, "ln2b", [4, 5, 6, 7], dma_tp=True)
        ffn1_tiles(1, list(range(NT)))
        for j in range(4, 8):
            ffn2_tile(j)

        for pool in (psum, work, attn, data, consts):
            pool.release()

    nc.compile()
    return nc


def _prep_inputs(x, Wq, Wk, Wv, Wp, bp, W1, b1, W2, b2, g1, be1, g2, be2):
    """Host-side preprocessing: fold LN affines into the following matmuls,
    pad per-head weights to 32-wide blocks, cast to bf16."""
    f32 = np.float32
    x = np.asarray(x, f32)
    Wqf = np.asarray(Wq, f32).reshape(C, C) * np.asarray(g1, f32)[:, None]
    Wkf = np.asarray(Wk, f32).reshape(C, C) * np.asarray(g1, f32)[:, None]
    Wvf = np.asarray(Wv, f32).reshape(C, C) * np.asarray(g1, f32)[:, None]
    bqf = np.asarray(be1, f32) @ np.asarray(Wq, f32).reshape(C, C)
    bkf = np.asarray(be1, f32) @ np.asarray(Wk, f32).reshape(C, C)
    bvf = np.asarray(be1, f32) @ np.asarray(Wv, f32).reshape(C, C)

    def pad_cols(w):
        wp = np.zeros((C, CP), f32)
        for h in range(H):
            wp[:, HP * h : HP * h + D] = w[:, D * h : D * (h + 1)]
        return wp

    def pad_vec(v):
        vp = np.zeros((CP,), f32)
        for h in range(H):
            vp[HP * h : HP * h + D] = v[D * h : D * (h + 1)]
        return vp

    wq_p = pad_cols(Wqf)
    wk_p = pad_cols(Wkf)
    wv_p = pad_cols(Wvf)
    bq_p = pad_vec(bqf)
    bk_p = pad_vec(bkf)
    bv_p = pad_vec(bvf)

    wp_p = np.zeros((CP, C), f32)
    for h in range(H):
        wp_p[HP * h : HP * h + D, :] = np.asarray(Wp, f32)[D * h : D * (h + 1), :]

    W1f = np.asarray(W1, f32) * np.asarray(g2, f32)[:, None]
    b1f = np.asarray(b1, f32) + np.asarray(be2, f32) @ np.asarray(W1, f32)

    shared = {
        "wq": wq_p.astype(_BF16), "wk": wk_p.astype(_BF16),
        "wv": wv_p.astype(_BF16), "wp": wp_p.astype(_BF16),
        "w1": W1f.astype(_BF16), "w2": np.asarray(W2, f32).astype(_BF16),
        "bq": bq_p, "bk": bk_p,
        "bprow": np.asarray(bp, f32), "b1p": b1f,
        "b2row": np.asarray(b2, f32),
    }
    assert not np.any(bv_p), "nonzero V bias not folded on-device (be1 != 0)"
    return x, shared


def kernel(**inputs) -> np.ndarray:
    from concourse import bass_utils

    x, shared = _prep_inputs(**inputs)
    qk_bias = bool(np.any(shared["bq"]) or np.any(shared["bk"]))
    bp_zero = not np.any(shared["bprow"])
    b2_zero = not np.any(shared["b2row"])
    key = ("nc", qk_bias, bp_zero, b2_zero)
    if key not in _cache:
        _cache[key] = _build_program(
            qk_bias=qk_bias, bp_zero=bp_zero, b2_zero=b2_zero)
    nc = _cache[key]

    in_maps = [dict(shared, x=np.ascontiguousarray(x[i])) for i in range(B)]
    res = bass_utils.run_bass_kernel_spmd(nc, in_maps, core_ids=list(range(B)))
    _cache["last_result"] = res
    out = np.stack([r["out"] for r in res.results], axis=0)
    return out.astype(np.float32)


# revision 25
# speedup vs baseline: 1.0194x; 1.0194x over previous
"""Trainium2 Bass kernel for one pre-LN transformer block (B=8, T=1024, C=256,
H=16 heads of size 16, FFN 256->1024->256), data-parallel over batch across 8
NeuronCores (one batch element per core).

v2 changes vs baseline (175-205us):
  - causal mask applied as a bitwise-AND on the bf16 *bits* of the exp'd
    scores (i32-packed keep-mask), one DVE op per diag tile covering all 4
    heads -- replaces the per-head strided multiply pass
  - exp evacuation split between ScalarE (true Exp) and VectorE (Schraudolph
    bits) per head-pair via a greedy runtime-cost balancer; all other PSUM
    evacuations (QKV copies, FFN1 relu, transpose copies) also greedy-balanced
    across the two PSUM-capable engines
  - PSUM tags: sps(4 banks) / pv(2) / mm(2, everything else) so attention and
    FFN never contend for banks
  - x DMA'd per-tile so LN1 starts after the first 128 rows land; weight DMAs
    moved to the gpsimd queue in first-use order
  - attention chunk-1 units interleaved between FFN chunk-0 stages to keep the
    PE array busy (HAM clock gate) through the whole span
"""

import os
import sys

for _p in ("/opt/trn_rl_repo", "/root/.axon_site/_ro/trn_rl_repo"):
    if os.path.isdir(_p) and _p not in sys.path:
        sys.path.append(_p)

import numpy as np
import ml_dtypes

# problem shapes (hardcoded per contest rules)
B, T, C, H, D, F = 8, 1024, 256, 16, 16, 1024
P = 128          # partitions
NT = T // P      # 8 T-tiles
HP = 32          # padded per-head stride (Q/K/V/out layouts)
CP = H * HP      # 512 padded channel dim
NPACK = 4        # head packs (4 heads per 128-partition tile)
NKC = C // P     # 2 k-tiles over C
EPS = 1e-5
SCALE = D ** -0.5
MAGIC = 0x5F3759DF
# Schraudolph-style exp to bf16 bits: bf16_bits(exp(SCALE*s)) ~= EXP_A*s + EXP_B
EXP_A = (2 ** 7) * SCALE * 1.4426950408889634
EXP_B = 2 ** 7 * 127 - 5.6

_BF16 = ml_dtypes.bfloat16

_cache = {}


def _build_program(qk_bias=False, bp_zero=False, b2_zero=False):
    import concourse.bass as bass
    import concourse.bacc as bacc
    import concourse.tile as tile
    import concourse.mybir as mybir

    dt = mybir.dt
    f32, bf16, i32, i16 = dt.float32, dt.bfloat16, dt.int32, dt.int16
    AF = mybir.ActivationFunctionType
    ALU = mybir.AluOpType

    nc = bacc.Bacc("TRN2", target_bir_lowering=False, debug=False)

    # ---- DRAM I/O ----
    x_d = nc.dram_tensor("x", [T, C], f32, kind="ExternalInput")
    wq_d = nc.dram_tensor("wq", [C, CP], bf16, kind="ExternalInput")
    wk_d = nc.dram_tensor("wk", [C, CP], bf16, kind="ExternalInput")
    wv_d = nc.dram_tensor("wv", [C, CP], bf16, kind="ExternalInput")
    wp_d = nc.dram_tensor("wp", [CP, C], bf16, kind="ExternalInput")
    w1_d = nc.dram_tensor("w1", [C, F], bf16, kind="ExternalInput")
    w2_d = nc.dram_tensor("w2", [F, C], bf16, kind="ExternalInput")
    bq_d = nc.dram_tensor("bq", [CP], f32, kind="ExternalInput")
    bk_d = nc.dram_tensor("bk", [CP], f32, kind="ExternalInput")
    bp_d = nc.dram_tensor("bprow", [C], f32, kind="ExternalInput")
    b1_d = nc.dram_tensor("b1p", [F], f32, kind="ExternalInput")
    b2_d = nc.dram_tensor("b2row", [C], f32, kind="ExternalInput")
    out_d = nc.dram_tensor("out", [T, C], f32, kind="ExternalOutput")

    ident_np = np.eye(P, dtype=_BF16)
    # causal keep-mask for a diagonal S^T tile, packed as i32 over bf16 pairs:
    # partition = tk local, i32 word w covers tq columns 2w (low16) / 2w+1
    # (high16); keep (all-ones halfword) iff tq >= tk.
    keep = (np.arange(P)[None, :] >= np.arange(P)[:, None])  # [tk, tq]
    lo = keep[:, 0::2].astype(np.uint32) * 0x0000FFFF
    hi = keep[:, 1::2].astype(np.uint32) * 0xFFFF0000
    triw_np = (lo | hi).astype(np.uint32).view(np.int32)  # [128, 64]
    triw4_np = np.tile(triw_np[:, None, :], (1, NPACK, 1))  # [128, 4, 64]
    trib4_np = np.tile(
        keep.astype(_BF16)[:, None, :], (1, NPACK, 1))  # [128, 4, 128] bf16
    ident_d = nc.inline_tensor(ident_np, name="ident")
    triw4_d = nc.inline_tensor(triw4_np, name="triw4")
    trib4_d = nc.inline_tensor(trib4_np, name="trib4")

    # greedy engine-load balancer for PSUM-evacuation ops
    eng_t = {"sc": 0.0, "dv": 0.0}

    def pick_engine(fd):
        cs = 0.84 * fd + 300.0
        cd = 1.05 * fd + 130.0
        if eng_t["sc"] + cs <= eng_t["dv"] + cd:
            eng_t["sc"] += cs
            return "sc"
        eng_t["dv"] += cd
        return "dv"

    def dv_fixed(fd, ovh=130.0):
        eng_t["dv"] += 1.05 * fd + ovh

    with tile.TileContext(nc) as tc:
        consts = tc.alloc_tile_pool(name="consts", bufs=1)
        data = tc.alloc_tile_pool(name="data", bufs=1)
        attn = tc.alloc_tile_pool(name="attn", bufs=1)
        work = tc.alloc_tile_pool(name="work", bufs=4)
        psum = tc.alloc_tile_pool(name="psum", bufs=1, space="PSUM")

        # ---- persistent SBUF tensors ----
        ident_s = consts.tile([P, P], bf16)
        triw4_s = consts.tile([P, NPACK, 64], i32)
        trib4_s = consts.tile([P, NPACK, P], bf16)
        wq_s = consts.tile([P, NKC, CP], bf16)
        wk_s = consts.tile([P, NKC, CP], bf16)
        wv_s = consts.tile([P, NKC, CP], bf16)
        wp_s = consts.tile([P, NPACK, C], bf16)
        w1_s = consts.tile([P, NKC, F], bf16)
        w2_s = consts.tile([P, NT, C], bf16)
        bq_s = consts.tile([P, NPACK], f32)
        bk_s = consts.tile([P, NPACK], f32)
        b1_s = consts.tile([P, NT], f32)

        xs = data.tile([P, NT, C], f32)
        xbp = xs if bp_zero else data.tile([P, NT, C], f32)
        h1T = data.tile([P, NKC, T], bf16)
        QT = data.tile([P, NPACK, T], bf16)
        KT = data.tile([P, NPACK, T], bf16)
        Vv = data.tile([P, NT, CP], bf16)
        OUTT = data.tile([P, NPACK, T], bf16)
        x1 = data.tile([P, NT, C], f32)
        x1b = x1 if b2_zero else data.tile([P, NT, C], f32)
        h2T = data.tile([P, NKC, T], bf16)
        HT = data.tile([P, NT, F], bf16)

        # ---- input DMAs: x tiles first, round-robined over the three DMA
        # queues so the first LN tiles land asap; weights follow in use order
        x_r = x_d[:, :].rearrange("(j p) c -> p j c", p=P)
        xq = [nc.sync, nc.scalar, nc.gpsimd]
        for j in range(NT):
            xq[j % 3].dma_start(out=xs[:, j], in_=x_r[:, j])
        nc.sync.dma_start(out=ident_s, in_=ident_d[:, :])
        nc.scalar.dma_start(out=triw4_s, in_=triw4_d[:, :, :])
        nc.scalar.dma_start(out=trib4_s, in_=trib4_d[:, :, :])
        nc.gpsimd.dma_start(out=wq_s, in_=wq_d[:, :].rearrange("(k p) c -> p k c", p=P))
        nc.gpsimd.dma_start(out=wk_s, in_=wk_d[:, :].rearrange("(k p) c -> p k c", p=P))
        nc.gpsimd.dma_start(out=wv_s, in_=wv_d[:, :].rearrange("(k p) c -> p k c", p=P))
        nc.sync.dma_start(out=wp_s, in_=wp_d[:, :].rearrange("(k p) c -> p k c", p=P))
        nc.sync.dma_start(out=w1_s, in_=w1_d[:, :].rearrange("(k p) c -> p k c", p=P))
        nc.sync.dma_start(out=w2_s, in_=w2_d[:, :].rearrange("(k p) c -> p k c", p=P))
        if qk_bias:
            nc.sync.dma_start(out=bq_s, in_=bq_d[:].rearrange("(m p) -> p m", p=P))
            nc.sync.dma_start(out=bk_s, in_=bk_d[:].rearrange("(m p) -> p m", p=P))
        nc.sync.dma_start(out=b1_s, in_=b1_d[:].rearrange("(m p) -> p m", p=P))
        if not bp_zero:
            bp_b = bass.AP(tensor=bp_d, offset=0, ap=[[0, P], [1, C]])
            bpt = consts.tile([P, C], f32)
            nc.sync.dma_start(out=bpt, in_=bp_b)
            for j in range(NT):
                nc.sync.dma_start(out=xbp[:, j], in_=x_r[:, j])
                nc.vector.tensor_add(out=xbp[:, j], in0=xbp[:, j], in1=bpt)
        if not b2_zero:
            b2t = consts.tile([P, C], f32)
            b2_b = bass.AP(tensor=b2_d, offset=0, ap=[[0, P], [1, C]])
            nc.sync.dma_start(out=b2t, in_=b2_b)

        def evac_copy(dst, src, fd):
            """PSUM->SBUF copy on whichever engine is less loaded."""
            if pick_engine(fd) == "sc":
                nc.scalar.copy(dst, src)
            else:
                nc.vector.tensor_copy(dst, src)

        def ln_phase(src, dst_hT, tag, tiles, dma_tp=False):
            """LayerNorm the given tiles of src [128, 8, 256] f32 and write
            the transposed bf16 result into dst_hT [128, 2, 1024].
            dma_tp: transpose via the DMA xbar (no PE / PSUM / evac) --
            preferred mid-kernel when the PE array is busy."""
            nj = len(tiles)
            mvall = work.tile([P, nj, 2], f32, tag="mvall", name=f"mv_{tag}")
            for jx, j in enumerate(tiles):
                stats = work.tile([P, 6], f32, tag="stats")
                nc.vector.bn_stats(out=stats, in_=src[:, j])
                nc.vector.bn_aggr(out=mvall[:, jx], in_=stats)
            dv_fixed(nj * 256 + nj * 8, nj * 300.0)
            # rstd for all tiles: Quake rsqrt + 2 Newton steps (pure DVE)
            vpe = work.tile([P, nj], f32, tag="vpe", name=f"vpe_{tag}")
            nc.vector.tensor_scalar_add(out=vpe, in0=mvall[:, :, 1], scalar1=EPS)
            sh = work.tile([P, nj], i32, tag="rsq_sh")
            nc.vector.tensor_scalar(
                out=sh, in0=vpe.bitcast(i32), scalar1=1, scalar2=None,
                op0=ALU.logical_shift_right,
            )
            y0 = work.tile([P, nj], i32, tag="rsq_y0")
            nc.vector.tensor_scalar(
                out=y0, in0=sh, scalar1=-1, scalar2=MAGIC,
                op0=ALU.mult, op1=ALU.add,
            )
            y = y0.bitcast(f32)
            rsq = work.tile([P, nj], f32, tag="rsq", name=f"rsq_{tag}")
            tmp = work.tile([P, nj], f32, tag="rsq_tmp")
            for it in range(2):
                nc.vector.tensor_tensor(out=tmp, in0=y, in1=y, op=ALU.mult)
                nc.vector.tensor_tensor(out=tmp, in0=tmp, in1=vpe, op=ALU.mult)
                nc.vector.tensor_scalar(
                    out=tmp, in0=tmp, scalar1=-0.5, scalar2=1.5,
                    op0=ALU.mult, op1=ALU.add,
                )
                nc.vector.tensor_tensor(out=rsq, in0=tmp, in1=y, op=ALU.mult)
                y = rsq
            dv_fixed(nj * 9, 9 * 130.0)
            for jx, j in enumerate(tiles):
                hs = work.tile([P, C], bf16, tag="hstraight")
                nc.vector.tensor_scalar(
                    out=hs, in0=src[:, j],
                    scalar1=mvall[:, jx, 0:1], scalar2=rsq[:, jx : jx + 1],
                    op0=ALU.subtract, op1=ALU.mult,
                )
                dv_fixed(128)
                if dma_tp:
                    nc.sync.dma_start(
                        out=dst_hT[:, :, j * P : (j + 1) * P], in_=hs,
                        transpose=True,
                    )
                else:
                    tp = psum.tile([P, 2, P], bf16, tag="big", bufs=3)
                    nc.tensor.transpose(tp[:, 0], hs[:, 0:P], ident_s)
                    nc.tensor.transpose(tp[:, 1], hs[:, P : 2 * P], ident_s)
                    evac_copy(dst_hT[:, :, j * P : (j + 1) * P], tp, 256)

        # ---- Q^T / K^T (padded layout, bias folded in evac) ----
        def qk_chunk(c):
            for (name, w_s, b_s, dstT) in (("q", wq_s, bq_s, QT), ("k", wk_s, bk_s, KT)):
                for m in range(NPACK):
                    ps = psum.tile([P, 512], f32, tag="big", bufs=3)
                    for k in range(NKC):
                        nc.tensor.matmul(
                            ps,
                            lhsT=w_s[:, k, m * P : (m + 1) * P],
                            rhs=h1T[:, k, c * 512 : (c + 1) * 512],
                            start=(k == 0), stop=(k == NKC - 1),
                        )
                    dst = dstT[:, m, c * 512 : (c + 1) * 512]
                    if qk_bias:
                        nc.vector.tensor_scalar_add(
                            out=dst, in0=ps, scalar1=b_s[:, m : m + 1])
                        dv_fixed(512)
                    else:
                        evac_copy(dst, ps, 512)

        # ---- V (straight, padded 32-wide blocks; col 16 of each = ones) ----
        def v_tiles(js):
            for j in js:
                ps = psum.tile([P, 512], f32, tag="big", bufs=3)
                for k in range(NKC):
                    nc.tensor.matmul(
                        ps,
                        lhsT=h1T[:, k, j * P : (j + 1) * P],
                        rhs=wv_s[:, k, :],
                        start=(k == 0), stop=(k == NKC - 1),
                    )
                evac_copy(Vv[:, j, :], ps, 512)
            ones_cols = Vv.rearrange("p j (h e) -> p j h e", e=HP)[
                :, js[0] : js[-1] + 1, :, 16:17]
            nc.gpsimd.memset(ones_cols, 1.0)

        # ---- attention: unit = (tq-chunk, pack) ----
        pend = []

        def tick():
            if pend:
                pend.pop(0)()

        def attn_unit(p, cj):
            nt_c = 4 * cj + 4
            expc = attn.tile([P, NPACK, nt_c, 512], bf16, tag=f"expc{cj}",
                             bufs=2, name=f"expc{p}_{cj}")
            tiles = list(range(0, min(NT, 4 * cj + 4)))
            # S^T as 32x32 subarray tiles; 2 heads share one 2-bank psum tile;
            # exp evac engine chosen per head-pair by the load balancer
            def exp_evac(dst, src, fd, eng):
                if eng == "sc":
                    nc.scalar.activation(
                        out=dst, in_=src, func=AF.Exp, scale=SCALE)
                else:
                    nc.vector.tensor_scalar(
                        out=dst.bitcast(i16), in0=src,
                        scalar1=EXP_A, scalar2=EXP_B,
                        op0=ALU.mult, op1=ALU.add,
                    )

            state = {"pv": None}
            for i in tiles:
                off = max(0, P * i - 512 * cj)  # valid start within chunk
                n = 512 - off
                # head pairs: greedy engine pick with a small penalty for
                # putting both pairs of one i-tile on the same engine (keeps
                # the two exps concurrent without forcing a 50/50 load split)
                prev_eng = None
                for q in range(2):
                    sp = psum.tile([P, 2, 512], f32, tag="big", bufs=3,
                                   name=f"sp{p}_{cj}_{i}_{q}")
                    for e in range(2):
                        hh = 2 * q + e
                        nc.tensor.matmul(
                            sp[:, e, 0:n],
                            lhsT=KT[HP * hh : HP * (hh + 1), p,
                                    i * P : (i + 1) * P],
                            rhs=QT[HP * hh : HP * (hh + 1), p,
                                   512 * cj + off : 512 * cj + off + n],
                            start=True, stop=True,
                            tile_position=(HP * hh, 0),
                        )
                    if prev_eng is not None:
                        eng_t[prev_eng] += 500.0
                        eng = pick_engine(2 * n)
                        eng_t[prev_eng] -= 500.0
                    else:
                        eng = pick_engine(2 * n)
                    prev_eng = eng
                    exp_evac(expc[:, 2 * q : 2 * q + 2, i, off : off + n],
                             sp[:, :, 0:n], 2 * n, eng)
                # causal mask on the diagonal tile: 0/1 bf16 multiply on the
                # otherwise-idle GpSimd -- PV now runs a full unit later, so
                # the longer GpSimd latency is completely hidden
                if 4 * cj <= i < 4 * cj + 4:
                    od = P * i - 512 * cj
                    eb = expc[:, :, i, od : od + P]
                    nc.gpsimd.tensor_tensor(
                        out=eb, in0=eb, in1=trib4_s, op=ALU.mult)
                # let one deferred PV step of the previous unit run between
                # this unit's S-tiles to keep the PE array streaming
                if pend:
                    pend.pop(0)()

            # deferred PV + normalize: emitted as thunks interleaved into the
            # NEXT unit's S-phase (keeps PE busy, hides mask latency)
            last = max(tiles)

            def pv_step(i):
                off = max(0, P * i - 512 * cj)
                n = 512 - off
                if state["pv"] is None:
                    state["pv"] = psum.tile([P, 512], f32, tag="pv", bufs=2,
                                            name=f"pv{p}_{cj}")
                for hh in range(NPACK):
                    h = 4 * p + hh
                    nc.tensor.matmul(
                        state["pv"][HP * hh : HP * (hh + 1), off : off + n],
                        lhsT=Vv[:, i, HP * h : HP * (h + 1)],
                        rhs=expc[:, hh, i, off : off + n],
                        start=(i == 0), stop=(i == last),
                        tile_position=(0, HP * hh),
                        skip_group_check=True,
                    )

            def norm_step():
                pv = state["pv"]
                zbc = work.tile([P, 512], f32, tag="zbc", bufs=2)
                rz = work.tile([P, 512], f32, tag="rz", bufs=2)
                nc.vector.stream_shuffle(zbc, pv, mask=[16] * 32)
                nc.vector.reciprocal_approx_fast(out=rz, in_=zbc)
                nc.vector.tensor_tensor(
                    out=OUTT[:, p, 512 * cj : 512 * (cj + 1)], in0=pv,
                    in1=rz, op=ALU.mult,
                )
                dv_fixed(3 * 512, 3 * 200.0)

            return [lambda i=i: pv_step(i) for i in tiles] + [norm_step]

        def proj_tile(j):
            ps = psum.tile([P, C], f32, tag="big", bufs=3)
            for k in range(NPACK):
                nc.tensor.matmul(
                    ps,
                    lhsT=OUTT[:, k, j * P : (j + 1) * P],
                    rhs=wp_s[:, k, :],
                    start=(k == 0), stop=(k == NPACK - 1),
                )
            nc.vector.tensor_add(out=x1[:, j], in0=ps, in1=xbp[:, j])
            dv_fixed(256)
            if not b2_zero:
                nc.vector.tensor_add(out=x1b[:, j], in0=x1[:, j], in1=b2t)
                dv_fixed(256)

        def ffn1_tiles(c, fs):
            for f in fs:
                tick()
                ps = psum.tile([P, 512], f32, tag="big", bufs=3)
                for k in range(NKC):
                    nc.tensor.matmul(
                        ps,
                        lhsT=w1_s[:, k, f * P : (f + 1) * P],
                        rhs=h2T[:, k, c * 512 : (c + 1) * 512],
                        start=(k == 0), stop=(k == NKC - 1),
                    )
                dst = HT[:, f, c * 512 : (c + 1) * 512]
                if pick_engine(512) == "sc":
                    nc.scalar.activation(
                        out=dst, in_=ps, func=AF.Relu, bias=b1_s[:, f : f + 1])
                else:
                    nc.vector.tensor_scalar(
                        out=dst, in0=ps, scalar1=b1_s[:, f : f + 1], scalar2=0.0,
                        op0=ALU.add, op1=ALU.max,
                    )

        def ffn2_tile(j):
            ps = psum.tile([P, C], f32, tag="big", bufs=3)
            for f in range(NT):
                nc.tensor.matmul(
                    ps,
                    lhsT=HT[:, f, j * P : (j + 1) * P],
                    rhs=w2_s[:, f, :],
                    start=(f == 0), stop=(f == NT - 1),
                )
            outs = work.tile([P, C], f32, tag="outs", bufs=2)
            nc.vector.tensor_add(out=outs, in0=ps, in1=x1b[:, j])
            dv_fixed(256)
            nc.sync.dma_start(
                out=out_d[:, :].rearrange("(t p) c -> p t c", p=P)[:, j], in_=outs
            )

        # ---- schedule: attention units start as soon as their inputs
        # exist and stay interleaved with LN/QKV/FFN so the exp engines are
        # never starved and the PE always has ready work ----
        ln_phase(xs, h1T, "ln1a0", [0, 1])
        ln_phase(xs, h1T, "ln1a1", [2, 3])
        qk_chunk(0)
        v_tiles([0, 1, 2, 3])
        ln_phase(xs, h1T, "ln1b0", [4, 5])
        ln_phase(xs, h1T, "ln1b1", [6, 7])
        def run_unit(p, cj):
            new_thunks = attn_unit(p, cj)
            pend.extend(new_thunks)

        run_unit(0, 0)
        qk_chunk(1)
        run_unit(1, 0)
        v_tiles([4, 5, 6, 7])
        run_unit(2, 0)
        run_unit(3, 0)
        run_unit(0, 1)
        for j in range(4):
            proj_tile(j)
            tick()
        run_unit(1, 1)
        ln_phase(x1, h2T, "ln2a", [0, 1, 2, 3], dma_tp=False)
        run_unit(2, 1)
        ffn1_tiles(0, [0, 1, 2, 3])
        run_unit(3, 1)
        ffn1_tiles(0, [4, 5, 6, 7])
        while pend:
            tick()
        for j in range(4, 8):
            proj_tile(j)
        for j in range(4):
            ffn2_tile(j)
        ln_phase(x1, h2T, "ln2b", [4, 5, 6, 7], dma_tp=False)
        ffn1_tiles(1, list(range(NT)))
        for j in range(4, 8):
            ffn2_tile(j)

        for pool in (psum, work, attn, data, consts):
            pool.release()

    nc.compile()
    return nc


def _prep_inputs(x, Wq, Wk, Wv, Wp, bp, W1, b1, W2, b2, g1, be1, g2, be2):
    """Host-side preprocessing: fold LN affines into the following matmuls,
    pad per-head weights to 32-wide blocks, cast to bf16."""
    f32 = np.float32
    x = np.asarray(x, f32)
    Wqf = np.asarray(Wq, f32).reshape(C, C) * np.asarray(g1, f32)[:, None]
    Wkf = np.asarray(Wk, f32).reshape(C, C) * np.asarray(g1, f32)[:, None]
    Wvf = np.asarray(Wv, f32).reshape(C, C) * np.asarray(g1, f32)[:, None]
    bqf = np.asarray(be1, f32) @ np.asarray(Wq, f32).reshape(C, C)
    bkf = np.asarray(be1, f32) @ np.asarray(Wk, f32).reshape(C, C)
    bvf = np.asarray(be1, f32) @ np.asarray(Wv, f32).reshape(C, C)

    def pad_cols(w):
        wp = np.zeros((C, CP), f32)
        for h in range(H):
            wp[:, HP * h : HP * h + D] = w[:, D * h : D * (h + 1)]
        return wp

    def pad_vec(v):
        vp = np.zeros((CP,), f32)
        for h in range(H):
            vp[HP * h : HP * h + D] = v[D * h : D * (h + 1)]
        return vp

    wq_p = pad_cols(Wqf)
    wk_p = pad_cols(Wkf)
    wv_p = pad_cols(Wvf)
    bq_p = pad_vec(bqf)
    bk_p = pad_vec(bkf)
    bv_p = pad_vec(bvf)

    wp_p = np.zeros((CP, C), f32)
    for h in range(H):
        wp_p[HP * h : HP * h + D, :] = np.asarray(Wp, f32)[D * h : D * (h + 1), :]

    W1f = np.asarray(W1, f32) * np.asarray(g2, f32)[:, None]
    b1f = np.asarray(b1, f32) + np.asarray(be2, f32) @ np.asarray(W1, f32)

    shared = {
        "wq": wq_p.astype(_BF16), "wk": wk_p.astype(_BF16),
        "wv": wv_p.astype(_BF16), "wp": wp_p.astype(_BF16),
        "w1": W1f.astype(_BF16), "w2": np.asarray(W2, f32).astype(_BF16),
        "bq": bq_p, "bk": bk_p,
        "bprow": np.asarray(bp, f32), "b1p": b1f,
        "b2row": np.asarray(b2, f32),
    }
    assert not np.any(bv_p), "nonzero V bias not folded on-device (be1 != 0)"
    return x, shared


def kernel(**inputs) -> np.ndarray:
    from concourse import bass_utils

    x, shared = _prep_inputs(**inputs)
    qk_bias = bool(np.any(shared["bq"]) or np.any(shared["bk"]))
    bp_zero = not np.any(shared["bprow"])
    b2_zero = not np.any(shared["b2row"])
    key = ("nc", qk_bias, bp_zero, b2_zero)
    if key not in _cache:
        _cache[key] = _build_program(
            qk_bias=qk_bias, bp_zero=bp_zero, b2_zero=b2_zero)
    nc = _cache[key]

    in_maps = [dict(shared, x=np.ascontiguousarray(x[i])) for i in range(B)]
    res = bass_utils.run_bass_kernel_spmd(nc, in_maps, core_ids=list(range(B)))
    _cache["last_result"] = res
    out = np.stack([r["out"] for r in res.results], axis=0)
    return out.astype(np.float32)


# revision 27
# speedup vs baseline: 1.0196x; 1.0002x over previous
"""Trainium2 Bass kernel for one pre-LN transformer block (B=8, T=1024, C=256,
H=16 heads of size 16, FFN 256->1024->256), data-parallel over batch across 8
NeuronCores (one batch element per core).

Changes vs the 175-205us baseline (measured ~143us):
  - exp evacuation split between ScalarE (true Exp) and VectorE (Schraudolph
    bf16-bits) per head-pair via a greedy build-time cost balancer with a
    same-engine penalty, so the two pairs of an i-tile exp concurrently; all
    other PSUM evacuations (QKV copies, FFN1 relu, transpose copies) are also
    greedy-balanced across the two PSUM-capable engines
  - causal mask as a 0/1 bf16 multiply on the otherwise-idle GpSimd engine
  - PV + normalize of each attention unit deferred into thunks drained
    one-per-i-tile inside the NEXT unit's S-phase (software pipelining: the
    PE array alternates S and PV work with no phase gap, and the GpSimd mask
    latency is fully hidden)
  - PSUM: one 3-slot [P,2,512] rotation ("big", 6 banks) shared by S-pairs,
    QKV/V/FFN/proj/transposes + a dedicated 2-bank "pv" rotation
  - per-chunk expc pools (chunk-0 units are half-sized) so chunk-0 and
    chunk-1 attention units can be in flight concurrently
  - x DMA'd per-tile round-robined over the sync/scalar/gpsimd queues so LN1
    starts as soon as tile 0 lands; weight DMAs follow in first-use order
  - schedule keeps ready attention work ahead of every DVE-bound phase
    boundary (LN2, proj) to avoid PE head-of-line stalls that re-throttle
    the HAM clock gate
"""

import os
import sys

for _p in ("/opt/trn_rl_repo", "/root/.axon_site/_ro/trn_rl_repo"):
    if os.path.isdir(_p) and _p not in sys.path:
        sys.path.append(_p)

import numpy as np
import ml_dtypes

# problem shapes (hardcoded per contest rules)
B, T, C, H, D, F = 8, 1024, 256, 16, 16, 1024
P = 128          # partitions
NT = T // P      # 8 T-tiles
HP = 32          # padded per-head stride (Q/K/V/out layouts)
CP = H * HP      # 512 padded channel dim
NPACK = 4        # head packs (4 heads per 128-partition tile)
NKC = C // P     # 2 k-tiles over C
EPS = 1e-5
SCALE = D ** -0.5
MAGIC = 0x5F3759DF
# Schraudolph-style exp to bf16 bits: bf16_bits(exp(SCALE*s)) ~= EXP_A*s + EXP_B
EXP_A = (2 ** 7) * SCALE * 1.4426950408889634
EXP_B = 2 ** 7 * 127 - 5.6

_BF16 = ml_dtypes.bfloat16

_cache = {}


def _build_program(qk_bias=False, bp_zero=False, b2_zero=False):
    import concourse.bass as bass
    import concourse.bacc as bacc
    import concourse.tile as tile
    import concourse.mybir as mybir

    dt = mybir.dt
    f32, bf16, i32, i16 = dt.float32, dt.bfloat16, dt.int32, dt.int16
    AF = mybir.ActivationFunctionType
    ALU = mybir.AluOpType

    nc = bacc.Bacc("TRN2", target_bir_lowering=False, debug=False)

    # ---- DRAM I/O ----
    x_d = nc.dram_tensor("x", [T, C], f32, kind="ExternalInput")
    wq_d = nc.dram_tensor("wq", [C, CP], bf16, kind="ExternalInput")
    wk_d = nc.dram_tensor("wk", [C, CP], bf16, kind="ExternalInput")
    wv_d = nc.dram_tensor("wv", [C, CP], bf16, kind="ExternalInput")
    wp_d = nc.dram_tensor("wp", [CP, C], bf16, kind="ExternalInput")
    w1_d = nc.dram_tensor("w1", [C, F], bf16, kind="ExternalInput")
    w2_d = nc.dram_tensor("w2", [F, C], bf16, kind="ExternalInput")
    bq_d = nc.dram_tensor("bq", [CP], f32, kind="ExternalInput")
    bk_d = nc.dram_tensor("bk", [CP], f32, kind="ExternalInput")
    bp_d = nc.dram_tensor("bprow", [C], f32, kind="ExternalInput")
    b1_d = nc.dram_tensor("b1p", [F], f32, kind="ExternalInput")
    b2_d = nc.dram_tensor("b2row", [C], f32, kind="ExternalInput")
    out_d = nc.dram_tensor("out", [T, C], f32, kind="ExternalOutput")

    ident_np = np.eye(P, dtype=_BF16)
    # causal keep-mask for a diagonal S^T tile, packed as i32 over bf16 pairs:
    # partition = tk local, i32 word w covers tq columns 2w (low16) / 2w+1
    # (high16); keep (all-ones halfword) iff tq >= tk.
    keep = (np.arange(P)[None, :] >= np.arange(P)[:, None])  # [tk, tq]
    lo = keep[:, 0::2].astype(np.uint32) * 0x0000FFFF
    hi = keep[:, 1::2].astype(np.uint32) * 0xFFFF0000
    triw_np = (lo | hi).astype(np.uint32).view(np.int32)  # [128, 64]
    triw4_np = np.tile(triw_np[:, None, :], (1, NPACK, 1))  # [128, 4, 64]
    trib4_np = np.tile(
        keep.astype(_BF16)[:, None, :], (1, NPACK, 1))  # [128, 4, 128] bf16
    ident_d = nc.inline_tensor(ident_np, name="ident")
    triw4_d = nc.inline_tensor(triw4_np, name="triw4")
    trib4_d = nc.inline_tensor(trib4_np, name="trib4")

    # greedy engine-load balancer for PSUM-evacuation ops
    eng_t = {"sc": 0.0, "dv": 0.0}

    def pick_engine(fd):
        cs = 0.84 * fd + 300.0
        cd = 1.05 * fd + 130.0
        if eng_t["sc"] + cs <= eng_t["dv"] + cd:
            eng_t["sc"] += cs
            return "sc"
        eng_t["dv"] += cd
        return "dv"

    def dv_fixed(fd, ovh=130.0):
        eng_t["dv"] += 1.05 * fd + ovh

    with tile.TileContext(nc) as tc:
        consts = tc.alloc_tile_pool(name="consts", bufs=1)
        data = tc.alloc_tile_pool(name="data", bufs=1)
        attn = tc.alloc_tile_pool(name="attn", bufs=1)
        work = tc.alloc_tile_pool(name="work", bufs=4)
        psum = tc.alloc_tile_pool(name="psum", bufs=1, space="PSUM")

        # ---- persistent SBUF tensors ----
        ident_s = consts.tile([P, P], bf16)
        triw4_s = consts.tile([P, NPACK, 64], i32)
        trib4_s = consts.tile([P, NPACK, P], bf16)
        wq_s = consts.tile([P, NKC, CP], bf16)
        wk_s = consts.tile([P, NKC, CP], bf16)
        wv_s = consts.tile([P, NKC, CP], bf16)
        wp_s = consts.tile([P, NPACK, C], bf16)
        w1_s = consts.tile([P, NKC, F], bf16)
        w2_s = consts.tile([P, NT, C], bf16)
        bq_s = consts.tile([P, NPACK], f32)
        bk_s = consts.tile([P, NPACK], f32)
        b1_s = consts.tile([P, NT], f32)

        xs = data.tile([P, NT, C], f32)
        xbp = xs if bp_zero else data.tile([P, NT, C], f32)
        h1T = data.tile([P, NKC, T], bf16)
        QT = data.tile([P, NPACK, T], bf16)
        KT = data.tile([P, NPACK, T], bf16)
        Vv = data.tile([P, NT, CP], bf16)
        OUTT = data.tile([P, NPACK, T], bf16)
        x1 = data.tile([P, NT, C], f32)
        x1b = x1 if b2_zero else data.tile([P, NT, C], f32)
        h2T = data.tile([P, NKC, T], bf16)
        HT = data.tile([P, NT, F], bf16)

        # ---- input DMAs: x tiles first, round-robined over the three DMA
        # queues so the first LN tiles land asap; weights follow in use order
        x_r = x_d[:, :].rearrange("(j p) c -> p j c", p=P)
        xq = [nc.sync, nc.scalar, nc.gpsimd]
        for j in range(NT):
            xq[j % 3].dma_start(out=xs[:, j], in_=x_r[:, j])
        nc.sync.dma_start(out=ident_s, in_=ident_d[:, :])
        nc.scalar.dma_start(out=triw4_s, in_=triw4_d[:, :, :])
        nc.scalar.dma_start(out=trib4_s, in_=trib4_d[:, :, :])
        nc.gpsimd.dma_start(out=wq_s, in_=wq_d[:, :].rearrange("(k p) c -> p k c", p=P))
        nc.gpsimd.dma_start(out=wk_s, in_=wk_d[:, :].rearrange("(k p) c -> p k c", p=P))
        nc.gpsimd.dma_start(out=wv_s, in_=wv_d[:, :].rearrange("(k p) c -> p k c", p=P))
        nc.sync.dma_start(out=wp_s, in_=wp_d[:, :].rearrange("(k p) c -> p k c", p=P))
        nc.sync.dma_start(out=w1_s, in_=w1_d[:, :].rearrange("(k p) c -> p k c", p=P))
        nc.sync.dma_start(out=w2_s, in_=w2_d[:, :].rearrange("(k p) c -> p k c", p=P))
        if qk_bias:
            nc.sync.dma_start(out=bq_s, in_=bq_d[:].rearrange("(m p) -> p m", p=P))
            nc.sync.dma_start(out=bk_s, in_=bk_d[:].rearrange("(m p) -> p m", p=P))
        nc.sync.dma_start(out=b1_s, in_=b1_d[:].rearrange("(m p) -> p m", p=P))
        if not bp_zero:
            bp_b = bass.AP(tensor=bp_d, offset=0, ap=[[0, P], [1, C]])
            bpt = consts.tile([P, C], f32)
            nc.sync.dma_start(out=bpt, in_=bp_b)
            for j in range(NT):
                nc.sync.dma_start(out=xbp[:, j], in_=x_r[:, j])
                nc.vector.tensor_add(out=xbp[:, j], in0=xbp[:, j], in1=bpt)
        if not b2_zero:
            b2t = consts.tile([P, C], f32)
            b2_b = bass.AP(tensor=b2_d, offset=0, ap=[[0, P], [1, C]])
            nc.sync.dma_start(out=b2t, in_=b2_b)

        def evac_copy(dst, src, fd):
            """PSUM->SBUF copy on whichever engine is less loaded."""
            if pick_engine(fd) == "sc":
                nc.scalar.copy(dst, src)
            else:
                nc.vector.tensor_copy(dst, src)

        def ln_phase(src, dst_hT, tag, tiles, dma_tp=False):
            """LayerNorm the given tiles of src [128, 8, 256] f32 and write
            the transposed bf16 result into dst_hT [128, 2, 1024].
            dma_tp: transpose via the DMA xbar (no PE / PSUM / evac) --
            preferred mid-kernel when the PE array is busy."""
            nj = len(tiles)
            mvall = work.tile([P, nj, 2], f32, tag="mvall", name=f"mv_{tag}")
            for jx, j in enumerate(tiles):
                stats = work.tile([P, 6], f32, tag="stats")
                nc.vector.bn_stats(out=stats, in_=src[:, j])
                nc.vector.bn_aggr(out=mvall[:, jx], in_=stats)
            dv_fixed(nj * 256 + nj * 8, nj * 300.0)
            # rstd for all tiles: Quake rsqrt + 2 Newton steps (pure DVE)
            vpe = work.tile([P, nj], f32, tag="vpe", name=f"vpe_{tag}")
            nc.vector.tensor_scalar_add(out=vpe, in0=mvall[:, :, 1], scalar1=EPS)
            sh = work.tile([P, nj], i32, tag="rsq_sh")
            nc.vector.tensor_scalar(
                out=sh, in0=vpe.bitcast(i32), scalar1=1, scalar2=None,
                op0=ALU.logical_shift_right,
            )
            y0 = work.tile([P, nj], i32, tag="rsq_y0")
            nc.vector.tensor_scalar(
                out=y0, in0=sh, scalar1=-1, scalar2=MAGIC,
                op0=ALU.mult, op1=ALU.add,
            )
            y = y0.bitcast(f32)
            rsq = work.tile([P, nj], f32, tag="rsq", name=f"rsq_{tag}")
            tmp = work.tile([P, nj], f32, tag="rsq_tmp")
            for it in range(2):
                nc.vector.tensor_tensor(out=tmp, in0=y, in1=y, op=ALU.mult)
                nc.vector.tensor_tensor(out=tmp, in0=tmp, in1=vpe, op=ALU.mult)
                nc.vector.tensor_scalar(
                    out=tmp, in0=tmp, scalar1=-0.5, scalar2=1.5,
                    op0=ALU.mult, op1=ALU.add,
                )
                nc.vector.tensor_tensor(out=rsq, in0=tmp, in1=y, op=ALU.mult)
                y = rsq
            dv_fixed(nj * 9, 9 * 130.0)
            for jx, j in enumerate(tiles):
                hs = work.tile([P, C], bf16, tag="hstraight")
                nc.vector.tensor_scalar(
                    out=hs, in0=src[:, j],
                    scalar1=mvall[:, jx, 0:1], scalar2=rsq[:, jx : jx + 1],
                    op0=ALU.subtract, op1=ALU.mult,
                )
                dv_fixed(128)
                if dma_tp:
                    nc.sync.dma_start(
                        out=dst_hT[:, :, j * P : (j + 1) * P], in_=hs,
                        transpose=True,
                    )
                else:
                    tp = psum.tile([P, 2, P], bf16, tag="big", bufs=3)
                    nc.tensor.transpose(tp[:, 0], hs[:, 0:P], ident_s)
                    nc.tensor.transpose(tp[:, 1], hs[:, P : 2 * P], ident_s)
                    evac_copy(dst_hT[:, :, j * P : (j + 1) * P], tp, 256)

        # ---- Q^T / K^T (padded layout, bias folded in evac) ----
        def qk_chunk(c):
            for (name, w_s, b_s, dstT) in (("q", wq_s, bq_s, QT), ("k", wk_s, bk_s, KT)):
                for m in range(NPACK):
                    ps = psum.tile([P, 512], f32, tag="big", bufs=3)
                    for k in range(NKC):
                        nc.tensor.matmul(
                            ps,
                            lhsT=w_s[:, k, m * P : (m + 1) * P],
                            rhs=h1T[:, k, c * 512 : (c + 1) * 512],
                            start=(k == 0), stop=(k == NKC - 1),
                        )
                    dst = dstT[:, m, c * 512 : (c + 1) * 512]
                    if qk_bias:
                        nc.vector.tensor_scalar_add(
                            out=dst, in0=ps, scalar1=b_s[:, m : m + 1])
                        dv_fixed(512)
                    else:
                        evac_copy(dst, ps, 512)

        # ---- V (straight, padded 32-wide blocks; col 16 of each = ones) ----
        def v_tiles(js):
            for j in js:
                ps = psum.tile([P, 512], f32, tag="big", bufs=3)
                for k in range(NKC):
                    nc.tensor.matmul(
                        ps,
                        lhsT=h1T[:, k, j * P : (j + 1) * P],
                        rhs=wv_s[:, k, :],
                        start=(k == 0), stop=(k == NKC - 1),
                    )
                evac_copy(Vv[:, j, :], ps, 512)
            ones_cols = Vv.rearrange("p j (h e) -> p j h e", e=HP)[
                :, js[0] : js[-1] + 1, :, 16:17]
            nc.gpsimd.memset(ones_cols, 1.0)

        # ---- attention: unit = (tq-chunk, pack) ----
        pend = []

        def tick():
            if pend:
                pend.pop(0)()

        def attn_unit(p, cj):
            nt_c = 4 * cj + 4
            expc = attn.tile([P, NPACK, nt_c, 512], bf16, tag=f"expc{cj}",
                             bufs=2, name=f"expc{p}_{cj}")
            tiles = list(range(0, min(NT, 4 * cj + 4)))
            # S^T as 32x32 subarray tiles; 2 heads share one 2-bank psum tile;
            # exp evac engine chosen per head-pair by the load balancer
            def exp_evac(dst, src, fd, eng):
                if eng == "sc":
                    nc.scalar.activation(
                        out=dst, in_=src, func=AF.Exp, scale=SCALE)
                else:
                    nc.vector.tensor_scalar(
                        out=dst.bitcast(i16), in0=src,
                        scalar1=EXP_A, scalar2=EXP_B,
                        op0=ALU.mult, op1=ALU.add,
                    )

            state = {"pv": None}
            for i in tiles:
                off = max(0, P * i - 512 * cj)  # valid start within chunk
                n = 512 - off
                # head pairs: greedy engine pick with a small penalty for
                # putting both pairs of one i-tile on the same engine (keeps
                # the two exps concurrent without forcing a 50/50 load split)
                prev_eng = None
                for q in range(2):
                    sp = psum.tile([P, 2, 512], f32, tag="big", bufs=3,
                                   name=f"sp{p}_{cj}_{i}_{q}")
                    for e in range(2):
                        hh = 2 * q + e
                        nc.tensor.matmul(
                            sp[:, e, 0:n],
                            lhsT=KT[HP * hh : HP * (hh + 1), p,
                                    i * P : (i + 1) * P],
                            rhs=QT[HP * hh : HP * (hh + 1), p,
                                   512 * cj + off : 512 * cj + off + n],
                            start=True, stop=True,
                            tile_position=(HP * hh, 0),
                        )
                    if prev_eng is not None:
                        eng_t[prev_eng] += 500.0
                        eng = pick_engine(2 * n)
                        eng_t[prev_eng] -= 500.0
                    else:
                        eng = pick_engine(2 * n)
                    prev_eng = eng
                    exp_evac(expc[:, 2 * q : 2 * q + 2, i, off : off + n],
                             sp[:, :, 0:n], 2 * n, eng)
                # causal mask on the diagonal tile: 0/1 bf16 multiply on the
                # otherwise-idle GpSimd -- PV now runs a full unit later, so
                # the longer GpSimd latency is completely hidden
                if 4 * cj <= i < 4 * cj + 4:
                    od = P * i - 512 * cj
                    eb = expc[:, :, i, od : od + P]
                    nc.gpsimd.tensor_tensor(
                        out=eb, in0=eb, in1=trib4_s, op=ALU.mult)
                # let one deferred PV step of the previous unit run between
                # this unit's S-tiles to keep the PE array streaming
                if pend:
                    pend.pop(0)()

            # deferred PV + normalize: emitted as thunks interleaved into the
            # NEXT unit's S-phase (keeps PE busy, hides mask latency)
            last = max(tiles)

            def pv_step(i):
                off = max(0, P * i - 512 * cj)
                n = 512 - off
                if state["pv"] is None:
                    state["pv"] = psum.tile([P, 512], f32, tag="pv", bufs=2,
                                            name=f"pv{p}_{cj}")
                for hh in range(NPACK):
                    h = 4 * p + hh
                    nc.tensor.matmul(
                        state["pv"][HP * hh : HP * (hh + 1), off : off + n],
                        lhsT=Vv[:, i, HP * h : HP * (h + 1)],
                        rhs=expc[:, hh, i, off : off + n],
                        start=(i == 0), stop=(i == last),
                        tile_position=(0, HP * hh),
                        skip_group_check=True,
                    )

            def norm_step():
                pv = state["pv"]
                zbc = work.tile([P, 512], f32, tag="zbc", bufs=2)
                rz = work.tile([P, 512], f32, tag="rz", bufs=2)
                nc.vector.stream_shuffle(zbc, pv, mask=[16] * 32)
                nc.vector.reciprocal_approx_fast(out=rz, in_=zbc)
                nc.vector.tensor_tensor(
                    out=OUTT[:, p, 512 * cj : 512 * (cj + 1)], in0=pv,
                    in1=rz, op=ALU.mult,
                )
                dv_fixed(3 * 512, 3 * 200.0)

            return [lambda i=i: pv_step(i) for i in tiles] + [norm_step]

        def proj_tile(j):
            ps = psum.tile([P, C], f32, tag="big", bufs=3)
            for k in range(NPACK):
                nc.tensor.matmul(
                    ps,
                    lhsT=OUTT[:, k, j * P : (j + 1) * P],
                    rhs=wp_s[:, k, :],
                    start=(k == 0), stop=(k == NPACK - 1),
                )
            nc.vector.tensor_add(out=x1[:, j], in0=ps, in1=xbp[:, j])
            dv_fixed(256)
            if not b2_zero:
                nc.vector.tensor_add(out=x1b[:, j], in0=x1[:, j], in1=b2t)
                dv_fixed(256)

        def ffn1_tiles(c, fs):
            for f in fs:
                tick()
                ps = psum.tile([P, 512], f32, tag="big", bufs=3)
                for k in range(NKC):
                    nc.tensor.matmul(
                        ps,
                        lhsT=w1_s[:, k, f * P : (f + 1) * P],
                        rhs=h2T[:, k, c * 512 : (c + 1) * 512],
                        start=(k == 0), stop=(k == NKC - 1),
                    )
                dst = HT[:, f, c * 512 : (c + 1) * 512]
                if pick_engine(512) == "sc":
                    nc.scalar.activation(
                        out=dst, in_=ps, func=AF.Relu, bias=b1_s[:, f : f + 1])
                else:
                    nc.vector.tensor_scalar(
                        out=dst, in0=ps, scalar1=b1_s[:, f : f + 1], scalar2=0.0,
                        op0=ALU.add, op1=ALU.max,
                    )

        def ffn2_tile(j):
            ps = psum.tile([P, C], f32, tag="big", bufs=3)
            for f in range(NT):
                nc.tensor.matmul(
                    ps,
                    lhsT=HT[:, f, j * P : (j + 1) * P],
                    rhs=w2_s[:, f, :],
                    start=(f == 0), stop=(f == NT - 1),
                )
            outs = work.tile([P, C], f32, tag="outs", bufs=2)
            nc.vector.tensor_add(out=outs, in0=ps, in1=x1b[:, j])
            dv_fixed(256)
            nc.sync.dma_start(
                out=out_d[:, :].rearrange("(t p) c -> p t c", p=P)[:, j], in_=outs
            )

        # ---- schedule: attention units start as soon as their inputs
        # exist and stay interleaved with LN/QKV/FFN so the exp engines are
        # never starved and the PE always has ready work ----
        ln_phase(xs, h1T, "ln1a0", [0, 1])
        ln_phase(xs, h1T, "ln1a1", [2, 3])
        qk_chunk(0)
        v_tiles([0, 1, 2, 3])
        ln_phase(xs, h1T, "ln1b0", [4, 5])
        ln_phase(xs, h1T, "ln1b1", [6, 7])
        def run_unit(p, cj):
            new_thunks = attn_unit(p, cj)
            pend.extend(new_thunks)

        run_unit(0, 0)
        qk_chunk(1)
        run_unit(1, 0)
        v_tiles([4, 5, 6, 7])
        run_unit(2, 0)
        run_unit(3, 0)
        run_unit(0, 1)
        for j in range(4):
            proj_tile(j)
            tick()
        run_unit(1, 1)
        ln_phase(x1, h2T, "ln2a", [0, 1, 2, 3], dma_tp=False)
        run_unit(2, 1)
        ffn1_tiles(0, [0, 1, 2, 3])
        run_unit(3, 1)
        ffn1_tiles(0, [4, 5, 6, 7])
        for j in range(4):
            ffn2_tile(j)
            tick()
            tick()
        while pend:
            tick()
        for j in range(4, 8):
            proj_tile(j)
        ln_phase(x1, h2T, "ln2b", [4, 5, 6, 7], dma_tp=False)
        ffn1_tiles(1, list(range(NT)))
        for j in range(4, 8):
            ffn2_tile(j)

        for pool in (psum, work, attn, data, consts):
            pool.release()

    nc.compile()
    return nc


def _prep_inputs(x, Wq, Wk, Wv, Wp, bp, W1, b1, W2, b2, g1, be1, g2, be2):
    """Host-side preprocessing: fold LN affines into the following matmuls,
    pad per-head weights to 32-wide blocks, cast to bf16."""
    f32 = np.float32
    x = np.asarray(x, f32)
    Wqf = np.asarray(Wq, f32).reshape(C, C) * np.asarray(g1, f32)[:, None]
    Wkf = np.asarray(Wk, f32).reshape(C, C) * np.asarray(g1, f32)[:, None]
    Wvf = np.asarray(Wv, f32).reshape(C, C) * np.asarray(g1, f32)[:, None]
    bqf = np.asarray(be1, f32) @ np.asarray(Wq, f32).reshape(C, C)
    bkf = np.asarray(be1, f32) @ np.asarray(Wk, f32).reshape(C, C)
    bvf = np.asarray(be1, f32) @ np.asarray(Wv, f32).reshape(C, C)

    def pad_cols(w):
        wp = np.zeros((C, CP), f32)
        for h in range(H):
            wp[:, HP * h : HP * h + D] = w[:, D * h : D * (h + 1)]
        return wp

    def pad_vec(v):
        vp = np.zeros((CP,), f32)
        for h in range(H):
            vp[HP * h : HP * h + D] = v[D * h : D * (h + 1)]
        return vp

    wq_p = pad_cols(Wqf)
    wk_p = pad_cols(Wkf)
    wv_p = pad_cols(Wvf)
    bq_p = pad_vec(bqf)
    bk_p = pad_vec(bkf)
    bv_p = pad_vec(bvf)

    wp_p = np.zeros((CP, C), f32)
    for h in range(H):
        wp_p[HP * h : HP * h + D, :] = np.asarray(Wp, f32)[D * h : D * (h + 1), :]

    W1f = np.asarray(W1, f32) * np.asarray(g2, f32)[:, None]
    b1f = np.asarray(b1, f32) + np.asarray(be2, f32) @ np.asarray(W1, f32)

    shared = {
        "wq": wq_p.astype(_BF16), "wk": wk_p.astype(_BF16),
        "wv": wv_p.astype(_BF16), "wp": wp_p.astype(_BF16),
        "w1": W1f.astype(_BF16), "w2": np.asarray(W2, f32).astype(_BF16),
        "bq": bq_p, "bk": bk_p,
        "bprow": np.asarray(bp, f32), "b1p": b1f,
        "b2row": np.asarray(b2, f32),
    }
    assert not np.any(bv_p), "nonzero V bias not folded on-device (be1 != 0)"
    return x, shared


def kernel(**inputs) -> np.ndarray:
    from concourse import bass_utils

    x, shared = _prep_inputs(**inputs)
    qk_bias = bool(np.any(shared["bq"]) or np.any(shared["bk"]))
    bp_zero = not np.any(shared["bprow"])
    b2_zero = not np.any(shared["b2row"])
    key = ("nc", qk_bias, bp_zero, b2_zero)
    if key not in _cache:
        _cache[key] = _build_program(
            qk_bias=qk_bias, bp_zero=bp_zero, b2_zero=b2_zero)
    nc = _cache[key]

    in_maps = [dict(shared, x=np.ascontiguousarray(x[i])) for i in range(B)]
    res = bass_utils.run_bass_kernel_spmd(nc, in_maps, core_ids=list(range(B)))
    _cache["last_result"] = res
    out = np.stack([r["out"] for r in res.results], axis=0)
    return out.astype(np.float32)
